# revision 6
# baseline (speedup 1.0000x reference)
"""Trainium2 Bass kernel for MultiHeadSelfAttention (RMSNorm + QKV + causal SDPA + out-proj).

Sharding: 8 cores = batch(2) x head-groups(4).  Each core handles one batch
element and 4 of the 16 heads; the out-projection is computed per-core over
its local 512-wide e-slice and the 4 partial [T, D] outputs per batch are
summed on the host.

v2 design (vs. the DRAM-roundtrip baseline):
  - Whole pipeline is SBUF-resident: q/k/v live in SBUF bf16, no DRAM bounce.
  - QKV projection runs in fp8(e4m3) DoubleRow with a 3-term hi/lo split
    (x = xh+xl, W = Wh+Wl; terms hh, hl, lh), 2 k-tiles per pass at 0.5
    cycles/row: 25% fewer PE cycles than bf16 at ~0.3% error.
  - Scores / AV / z run in bf16 (full rate at any N, so the causal diagonal
    is computed at its true 128-column cost).
  - Softmax denominator: off-diagonal pt tiles are quad-summed on DVE (bf16,
    2x mode) so the PE z-matmul only streams 1/4 of the columns.
  - RMSNorm stats are per-chunk: squares of x_hi8 on ACT/DVE, summed over d
    by a ones-matmul; the 1/rms scale and the fp8 unscale fold into the
    PSUM->SBUF moves of q/k (DVE) and v (ACT, per-partition scale vector).
  - Emission is software-pipelined: proj(c+1) runs on the PE while DVE/ACT
    drain attn(c) dependencies; outproj(2) interleaves into attn(3) heads.
"""

import sys

sys.path.insert(0, '/opt/trn_rl_repo')

import numpy as np
import ml_dtypes

import concourse.bass as bass  # noqa: F401  (import order matters)
from concourse import bacc
import concourse.mybir as mybir
import concourse.tile as tile
from concourse.bass_utils import run_bass_kernel_spmd

B, T, D = 2, 2048, 2048
H_LOC, DH = 4, 128
EL = H_LOC * DH            # 512: local q/k/v width
ND = D // 128              # 16 d-tiles
NT = T // 128              # 16 t-tiles
CH = 512                   # token chunk
NCH = T // CH              # 4 chunks
QT = CH // 128             # 4 q-tiles per chunk
EPS = 1e-6
SX = 16.0                  # x fp8 scale
SW = 512.0                 # w_qkv fp8 scale
UNS = 1.0 / (SX * SW)      # unscale for qkv projection results
SY = 16.0                  # y fp8 scale
SWO = 512.0                # w_out fp8 scale
UO = 1.0 / (SY * SWO)      # unscale for out-projection results
F32 = mybir.dt.float32
BF16 = mybir.dt.bfloat16
FP8 = mybir.dt.float8e4
MULT = mybir.AluOpType.mult
ADD = mybir.AluOpType.add
DR = mybir.MatmulPerfMode.DoubleRow
EXP = mybir.ActivationFunctionType.Exp
SQRT = mybir.ActivationFunctionType.Sqrt
COPY = mybir.ActivationFunctionType.Copy
SC = float(1.0 / np.sqrt(DH))


def _build():
    nc = bacc.Bacc("TRN2")
    xh8 = nc.dram_tensor("xh8", [D, T], FP8, kind="ExternalInput")
    xl8 = nc.dram_tensor("xl8", [D, T], FP8, kind="ExternalInput")
    xtd8 = nc.dram_tensor("xtd8", [T, D], FP8, kind="ExternalInput")
    wh8 = nc.dram_tensor("wh8", [D, 3 * EL], FP8, kind="ExternalInput")
    wl8 = nc.dram_tensor("wl8", [D, 3 * EL], FP8, kind="ExternalInput")
    wouth8 = nc.dram_tensor("wouth8", [EL, D], FP8, kind="ExternalInput")
    woutl8 = nc.dram_tensor("woutl8", [EL, D], FP8, kind="ExternalInput")
    mask_in = nc.dram_tensor("mask_in", [128, 128], BF16, kind="ExternalInput")
    ones_in = nc.dram_tensor("ones_in", [128, 8], BF16, kind="ExternalInput")
    outT = nc.dram_tensor("outT", [D, T], BF16, kind="ExternalOutput")

    xh_ap = xh8.ap().rearrange("(k p) t -> p k t", p=128)
    xl_ap = xl8.ap().rearrange("(k p) t -> p k t", p=128)
    wh_ap = wh8.ap().rearrange("(k p) e -> p k e", p=128)
    wl_ap = wl8.ap().rearrange("(k p) e -> p k e", p=128)
    woh_ap = wouth8.ap().rearrange("(k p) e -> p k e", p=128)
    wol_ap = woutl8.ap().rearrange("(k p) e -> p k e", p=128)
    xtd_ap = xtd8.ap().rearrange("(k p) e -> p k e", p=128)

    with tile.TileContext(nc) as tc:
        with tc.tile_pool(name="per", bufs=1) as per, \
             tc.tile_pool(name="xp", bufs=2) as xp, \
             tc.tile_pool(name="sqp", bufs=2) as sqp, \
             tc.tile_pool(name="tdp", bufs=1) as tdp, \
             tc.tile_pool(name="stt", bufs=2) as sttp, \
             tc.tile_pool(name="qp", bufs=2) as qp, \
             tc.tile_pool(name="ptp", bufs=6) as ptp, \
             tc.tile_pool(name="qdp", bufs=2) as qdp, \
             tc.tile_pool(name="fxp", bufs=2) as fxp, \
             tc.tile_pool(name="yp", bufs=2) as yp, \
             tc.tile_pool(name="op", bufs=6) as op_p, \
             tc.tile_pool(name="msc", bufs=2) as msc, \
             tc.tile_pool(name="dram", bufs=2, space="DRAM") as dramp, \
             tc.tile_pool(name="pjps", bufs=2, space="PSUM") as pjps, \
             tc.tile_pool(name="stps", bufs=2, space="PSUM") as stps, \
             tc.tile_pool(name="yps", bufs=1, space="PSUM") as yps, \
             tc.tile_pool(name="zps", bufs=1, space="PSUM") as zps:

            # ---------------- persistent SBUF tensors ----------------
            w_h = per.tile([128, ND, 3 * EL], FP8)
            w_l = per.tile([128, ND, 3 * EL], FP8)
            wout_h = per.tile([128, H_LOC, D], FP8)
            wout_l = per.tile([128, H_LOC, D], FP8)
            k_sb = per.tile([128, H_LOC, T], BF16)
            v_sb = per.tile([128, NT, EL], BF16)
            mask_sb = per.tile([128, 128], BF16)
            ones_sb = per.tile([128, 8], BF16)
            rS_col = per.tile([128, ND], F32)
            eps_sb = per.tile([128, 1], F32)

            x_tiles = {}

            def load_x(c, split=False):
                xt_h = xp.tile([128, ND, CH], FP8, tag="xh", name=f"xh_{c}")
                xt_l = xp.tile([128, ND, CH], FP8, tag="xl", name=f"xl_{c}")
                cs = slice(c * CH, (c + 1) * CH)
                if split:
                    nc.sync.dma_start(xt_h[:, 0:8, :], xh_ap[:, 0:8, cs])
                    nc.sync.dma_start(mask_sb[:], mask_in[:, :])
                    nc.sync.dma_start(ones_sb[:], ones_in[:, :])
                    nc.sync.dma_start(xt_h[:, 8:16, :], xh_ap[:, 8:16, cs])
                    nc.sync.dma_start(xt_l[:, 0:8, :], xl_ap[:, 0:8, cs])
                    nc.sync.dma_start(xt_l[:, 8:16, :], xl_ap[:, 8:16, cs])
                else:
                    nc.sync.dma_start(xt_h[:], xh_ap[:, :, cs])
                    nc.sync.dma_start(xt_l[:], xl_ap[:, :, cs])
                x_tiles[c] = (xt_h, xt_l)

            # DMA order: x0_hi first (stats + first proj term), then weight
            # groups interleaved with x0_lo so group-major units stream.
            xt_h0 = xp.tile([128, ND, CH], FP8, tag="xh", name="xh_0")
            xt_l0 = xp.tile([128, ND, CH], FP8, tag="xl", name="xl_0")
            x_tiles[0] = (xt_h0, xt_l0)
            nc.sync.dma_start(w_h[:, 0:2, :], wh_ap[:, 0:2, :])
            nc.sync.dma_start(xt_h0[:, 0:8, :], xh_ap[:, 0:8, 0:CH])
            nc.sync.dma_start(mask_sb[:], mask_in[:, :])
            nc.sync.dma_start(ones_sb[:], ones_in[:, :])
            nc.sync.dma_start(xt_h0[:, 8:16, :], xh_ap[:, 8:16, 0:CH])
            nc.gpsimd.memset(eps_sb[:], EPS)

            sb_tiles = {}

            def stats(c):
                """Per-chunk RMS stats from token-major fp8 x: square+accum
                on ACT/DVE (no PE work), then sqrt/recip in column form and a
                DMA transpose to row form for the q/k scale broadcast."""
                td = tdp.tile([128, QT, D], FP8, tag="td", name=f"td_{c}")
                nc.sync.dma_start(td[:], xtd_ap[:, c * QT:(c + 1) * QT, :])
                ssq4 = sttp.tile([128, QT], F32, tag="ssq4")
                for tt in range(QT):
                    sq = sqp.tile([128, D], FP8, tag="sq")
                    if tt % 2 == 0:
                        nc.scalar.activation(sq[:], td[:, tt, :], mybir.ActivationFunctionType.Square,
                                             scale=1.0 / SX, accum_out=ssq4[:, tt:tt + 1])
                    else:
                        nc.vector.scalar_tensor_tensor(sq[:], td[:, tt, :], 1.0 / (SX * SX), td[:, tt, :],
                                                       MULT, MULT, accum_out=ssq4[:, tt:tt + 1])
                cs4 = slice(c * QT, (c + 1) * QT)
                rms4 = sttp.tile([128, QT], F32, tag="rms4")
                nc.scalar.activation(rms4[:], ssq4[:], SQRT,
                                     bias=eps_sb[:], scale=1.0 / D)
                nc.vector.reciprocal(rS_col[:, cs4], rms4[:])
                nc.vector.tensor_scalar_mul(rS_col[:, cs4], rS_col[:, cs4], UNS)
                rcol_d = dramp.tile([QT, 128], F32, tag="rrd", name=f"rrd_{c}")
                nc.sync.dma_start(rcol_d[:, :].rearrange("j p -> p j"), rS_col[:, cs4])
                rS_row = sttp.tile([1, CH], F32, tag="rsr", bufs=1)
                nc.sync.dma_start(rS_row[0:1, :], rcol_d[:, :].rearrange("(o j) p -> o (j p)", o=1))
                sb_c = sttp.tile([128, CH], F32, tag="sbc")
                nc.gpsimd.partition_broadcast(sb_c[:], rS_row[:])
                sb_tiles[c] = sb_c

            # ---------------- building blocks ----------------
            def _adv(filler, n=1):
                if filler is None:
                    return
                for _ in range(n):
                    try:
                        next(filler)
                    except StopIteration:
                        return

            def _flush(filler):
                if filler is None:
                    return
                for _ in filler:
                    pass

            def proj_unit(c, u, wide=False, terms='AB'):
                """One output M-tile of the fused QKV projection for chunk c.
                u in 0..7 -> q/k e-tiles, u in 8..11 -> v t-tiles.  wide=True
                borrows the idle attention PSUM pools (chunk 0 only) so more
                units can accumulate while weight groups stream in.  terms:
                'AB' all three; 'A' = wh terms only; 'B' = deferred wl term
                accumulated into the destination via an additive fixup."""
                xt_h, xt_l = x_tiles[c]
                if wide:
                    r = u % 4
                    if r < 2:
                        ps = pjps.tile([128, CH], F32, tag="pj")
                    elif r == 2:
                        ps_w = stps.tile([128, 2, CH], F32, tag="st", name=f"pw_{u}")
                        ps = ps_w[:, 0, :]
                    else:
                        ps = yps.tile([128, CH], F32, tag="y", name=f"py_{u}")
                else:
                    ps = pjps.tile([128, CH], F32, tag="pj")
                if terms == 'A':
                    tl = ((w_h, xt_h), (w_h, xt_l))
                elif terms == 'B':
                    tl = ((w_l, xt_h),)
                else:
                    tl = ((w_h, xt_h), (w_h, xt_l), (w_l, xt_h))
                nlast = 8 * len(tl) - 1
                if u < 8:
                    es = slice(u * 128, (u + 1) * 128)
                    n = 0
                    for g4 in range(4):
                        for wt, xt in tl:
                            for kp in range(2 * g4, 2 * g4 + 2):
                                kk = slice(2 * kp, 2 * kp + 2)
                                nc.tensor.matmul(ps[:], wt[:, kk, es], xt[:, kk, :],
                                                 start=(n == 0), stop=(n == nlast),
                                                 perf_mode=DR)
                                n += 1
                    # scale by r/(SX*SW) per token column, cast to bf16
                    if u < 4:
                        dst = q_tiles[c][:, u, :]
                    else:
                        dst = k_sb[:, u - 4, c * CH:(c + 1) * CH]
                    if terms == 'B':
                        fx = fxp.tile([128, CH], BF16, tag="fx")
                        nc.vector.tensor_tensor(fx[:], ps[:], sb_tiles[c][:], MULT)
                        nc.vector.tensor_tensor(dst, dst, fx[:], ADD)
                    else:
                        nc.vector.tensor_tensor(dst, ps[:], sb_tiles[c][:], MULT)
                else:
                    tt = u - 8
                    j = c * QT + tt
                    ts = slice(tt * 128, (tt + 1) * 128)
                    vs = slice(2 * EL, 3 * EL)
                    n = 0
                    for g4 in range(4):
                        for wt, xt in tl:
                            for kp in range(2 * g4, 2 * g4 + 2):
                                kk = slice(2 * kp, 2 * kp + 2)
                                nc.tensor.matmul(ps[:], xt[:, kk, ts], wt[:, kk, vs],
                                                 start=(n == 0), stop=(n == nlast),
                                                 perf_mode=DR)
                                n += 1
                    if terms == 'B':
                        fx = fxp.tile([128, CH], BF16, tag="fx")
                        nc.vector.tensor_scalar_mul(fx[:], ps[:], rS_col[:, j:j + 1])
                        nc.vector.tensor_tensor(v_sb[:, j, :], v_sb[:, j, :], fx[:], ADD)
                    else:
                        nc.scalar.activation(v_sb[:, j, :], ps[:], COPY,
                                             scale=rS_col[:, j:j + 1])

            def emit_proj(c, wide=False):
                q_tiles[c] = qp.tile([128, H_LOC, CH], BF16, tag="q", name=f"q_{c}")
                for u in range(12):
                    proj_unit(c, u, wide=wide)

            def proj_steps(c):
                """Generator: proj(c) split into 8-matmul steps (yield between)."""
                q_tiles[c] = qp.tile([128, H_LOC, CH], BF16, tag="q", name=f"q_{c}")
                xt_h, xt_l = x_tiles[c]
                for u in range(12):
                    ps = pjps.tile([128, CH], F32, tag="pj")
                    if u < 8:
                        es = slice(u * 128, (u + 1) * 128)
                        ops = [(wt[:, :, es], xt, True) for wt, xt in
                               ((w_h, xt_h), (w_h, xt_l), (w_l, xt_h))]
                    else:
                        tt = u - 8
                        ts = slice(tt * 128, (tt + 1) * 128)
                        vs = slice(2 * EL, 3 * EL)
                        ops = [(xt[:, :, ts], wt, False) for xt, wt in
                               ((xt_h, w_h), (xt_l, w_h), (xt_h, w_l))]
                    n = 0
                    for a, bmat, qk in ops:
                        for kp in range(ND // 2):
                            kk = slice(2 * kp, 2 * kp + 2)
                            if qk:
                                nc.tensor.matmul(ps[:], a[:, kk, :], bmat[:, kk, :],
                                                 start=(n == 0), stop=(n == 23),
                                                 perf_mode=DR)
                            else:
                                nc.tensor.matmul(ps[:], a[:, kk, :],
                                                 bmat[:, kk, 2 * EL:3 * EL],
                                                 start=(n == 0), stop=(n == 23),
                                                 perf_mode=DR)
                            n += 1
                        yield
                    if u < 4:
                        nc.vector.tensor_tensor(q_tiles[c][:, u, :], ps[:], sb_tiles[c][:], MULT)
                    elif u < 8:
                        nc.vector.tensor_tensor(k_sb[:, u - 4, c * CH:(c + 1) * CH],
                                                ps[:], sb_tiles[c][:], MULT)
                    else:
                        j = c * QT + (u - 8)
                        nc.scalar.activation(v_sb[:, j, :], ps[:], COPY,
                                             scale=rS_col[:, j:j + 1])

            def outproj_steps(c, eos, copy_eng='alt', wide=False):
                """Generator: outproj units for chunk c, one unit per step."""
                y_sb = y_tiles[c]
                yh_sb, yl_sb = y_sb
                for eo in eos:
                    if wide:
                        r = eo % 4
                        if r < 2:
                            o_ps = pjps.tile([128, CH], F32, tag="pj")
                        elif r == 2:
                            o_w = stps.tile([128, 2, CH], F32, tag="st", name=f"ow_{c}_{eo}")
                            o_ps = o_w[:, 0, :]
                        else:
                            o_ps = yps.tile([128, CH], F32, tag="y", name=f"oy_{c}_{eo}")
                    else:
                        o_ps = pjps.tile([128, CH], F32, tag="pj")
                    es = slice(eo * 128, (eo + 1) * 128)
                    n = 0
                    for wt, yt in ((wout_h, yh_sb), (wout_h, yl_sb), (wout_l, yh_sb)):
                        for dp in range(H_LOC // 2):
                            dd = slice(2 * dp, 2 * dp + 2)
                            nc.tensor.matmul(o_ps[:], wt[:, dd, es], yt[:, dd, :],
                                             start=(n == 0), stop=(n == 5),
                                             perf_mode=DR)
                            n += 1
                        yield
                    o_sb = op_p.tile([128, CH], BF16, tag="o")
                    if (eo % 4 == 3 and c != 3) or (c == 3 and eo % 2 == 1):
                        nc.scalar.activation(o_sb[:], o_ps[:], COPY, scale=UO)
                    else:
                        nc.vector.tensor_scalar_mul(o_sb[:], o_ps[:], UO)
                    nc.sync.dma_start(outT[eo * 128:(eo + 1) * 128, c * CH:(c + 1) * CH], o_sb[:])
                    yield

            def chain(*gens):
                for g in gens:
                    for _ in g:
                        yield

            def attn_head(c, h, q_sb, y_sb, filler=None):
                """Causal attention for head h over q-chunk c."""
                jmax = (c + 1) * QT
                npair = c * QT // 2          # off-diagonal tile pairs
                pt_tiles = []                # (tile, off) per j
                # --- scores + exp: off-diagonal pairs ---
                for pp in range(npair):
                    st = stps.tile([128, 2, CH], F32, tag="st")
                    for i in range(2):
                        j = 2 * pp + i
                        nc.tensor.matmul(st[:, i, :], k_sb[:, h, j * 128:(j + 1) * 128],
                                         q_sb[:, h, :], start=True, stop=True)
                    pt = ptp.tile([128, 2, CH], BF16, tag="pt")
                    nc.scalar.activation(pt[:, :, :].rearrange("p a b -> p (a b)"),
                                         st[:, :, :].rearrange("p a b -> p (a b)"),
                                         EXP, scale=SC)
                    pt_tiles.append((pt, 0))
                    _adv(filler)
                # --- diagonal tiles ---
                for dt_i in range(QT):
                    j = c * QT + dt_i
                    off = dt_i * 128
                    st = stps.tile([128, 2, CH], F32, tag="st")
                    nc.tensor.matmul(st[:, 0, off:], k_sb[:, h, j * 128:(j + 1) * 128],
                                     q_sb[:, h, off:], start=True, stop=True)
                    pt = ptp.tile([128, CH], BF16, tag="ptd", bufs=4)
                    nc.scalar.activation(pt[:, off:], st[:, 0, off:], EXP, scale=SC)
                    nc.vector.tensor_tensor(pt[:, off:off + 128], pt[:, off:off + 128],
                                            mask_sb[:], MULT)
                    pt_tiles.append((pt, off))
                    if dt_i % 2 == 1:
                        _adv(filler)
                # --- z: quad-sums for pairs, direct for diagonal ---
                z = zps.tile([1, CH], F32, tag="zrow")
                nz = 0
                nzt = (npair + 1) // 2 + QT
                for qq in range((npair + 1) // 2):
                    pa, _ = pt_tiles[2 * qq]
                    if 2 * qq + 1 < npair:
                        pb, _ = pt_tiles[2 * qq + 1]
                        ab = qdp.tile([128, 2, CH], BF16, tag="ab")
                        nc.vector.tensor_tensor(ab[:, :, :].rearrange("p a b -> p (a b)"),
                                                pa[:, :, :].rearrange("p a b -> p (a b)"),
                                                pb[:, :, :].rearrange("p a b -> p (a b)"), ADD)
                        qd = qdp.tile([128, CH], BF16, tag="qd", bufs=3)
                        nc.vector.tensor_tensor(qd[:], ab[:, 0, :], ab[:, 1, :], ADD)
                    else:
                        qd = qdp.tile([128, CH], BF16, tag="qd", bufs=3)
                        nc.vector.tensor_tensor(qd[:], pa[:, 0, :], pa[:, 1, :], ADD)
                    nc.tensor.matmul(z[:], ones_sb[:, 0:1], qd[:],
                                     start=(nz == 0), stop=(nz == nzt - 1))
                    nz += 1
                for dt_i in range(QT):
                    pt, off = pt_tiles[npair + dt_i]
                    nc.tensor.matmul(z[0:1, off:], ones_sb[:, 0:1], pt[:, off:],
                                     start=(nz == 0), stop=(nz == nzt - 1))
                    nz += 1
                _adv(filler)
                # --- AV ---
                y_ps = yps.tile([128, CH], F32, tag="y")
                for j in range(jmax):
                    if j % 4 == 3:
                        _adv(filler)
                    if j < npair * 2:
                        pt, _ = pt_tiles[j // 2]
                        src = pt[:, j % 2, :]
                        off = 0
                    else:
                        pt, off = pt_tiles[npair + (j - npair * 2)]
                        src = pt[:, off:]
                    nc.tensor.matmul(y_ps[:, off:], v_sb[:, j, h * 128:(h + 1) * 128],
                                     src, start=(j == 0), stop=(j == jmax - 1))
                # --- normalize + fp8 hi/lo split (scaled by SY) ---
                yh_sb, yl_sb = y_sb
                rz = msc.tile([1, CH], F32, tag="rz")
                nc.vector.reciprocal(rz[:], z[:])
                nc.vector.tensor_scalar_mul(rz[:], rz[:], SY)
                rzb = msc.tile([128, CH], F32, tag="rzb")
                nc.gpsimd.partition_broadcast(rzb[:], rz[:])
                ytmp = msc.tile([128, CH], F32, tag="ytmp")
                nc.vector.tensor_tensor(ytmp[:], y_ps[:], rzb[:], MULT)
                nc.vector.tensor_copy(yh_sb[:, h, :], ytmp[:])
                nc.vector.scalar_tensor_tensor(yl_sb[:, h, :], ytmp[:], 1.0, yh_sb[:, h, :],
                                               MULT, mybir.AluOpType.subtract)

            def emit_attn(c, filler=None):
                yh = yp.tile([128, H_LOC, CH], FP8, tag="yh", name=f"yh_{c}")
                yl = yp.tile([128, H_LOC, CH], FP8, tag="yl", name=f"yl_{c}")
                y_tiles[c] = (yh, yl)
                for h in range(H_LOC):
                    attn_head(c, h, q_tiles[c], y_tiles[c], filler)
                _flush(filler)

            def proj_with_attn(pc, ac, filler=None):
                """proj(pc) with attn(ac) heads interleaved between units:
                the proj matmuls swallow the exp/DVE latency of each head."""
                yh = yp.tile([128, H_LOC, CH], FP8, tag="yh", name=f"yh_{ac}")
                yl = yp.tile([128, H_LOC, CH], FP8, tag="yl", name=f"yl_{ac}")
                y_tiles[ac] = (yh, yl)
                q_tiles[pc] = qp.tile([128, H_LOC, CH], BF16, tag="q", name=f"q_{pc}")
                hsched = {3: 0, 5: 1, 7: 2, 9: 3}
                for u in range(12):
                    proj_unit(pc, u)
                    if u in hsched:
                        attn_head(ac, hsched[u], q_tiles[ac], y_tiles[ac], filler)
                _flush(filler)

            def outproj_part(c, eos, copy_eng='alt', wide=False):
                for _ in outproj_steps(c, eos, copy_eng, wide):
                    pass

            # ---------------- software-pipelined emission ----------------
            q_tiles = {}
            y_tiles = {}

            nc.sync.dma_start(w_h[:, 2:4, :], wh_ap[:, 2:4, :])
            stats(0)
            nc.sync.dma_start(xt_l0[:, 0:8, :], xl_ap[:, 0:8, 0:CH])
            nc.sync.dma_start(xt_l0[:, 8:16, :], xl_ap[:, 8:16, 0:CH])
            for g4 in range(1, 4):
                s = slice(4 * g4, 4 * g4 + 4)
                nc.sync.dma_start(w_h[:, s, :], wh_ap[:, s, :])
            # --- proj(0) A-part: wh-only terms (hh + hl), streams with w_h ---
            q_tiles[0] = qp.tile([128, H_LOC, CH], BF16, tag="q", name="q_0")
            for u in range(12):
                proj_unit(0, u, wide=True, terms='A')
            for g4 in range(4):
                s = slice(4 * g4, 4 * g4 + 4)
                nc.sync.dma_start(w_l[:, s, :], wl_ap[:, s, :])
            stats(1)
            load_x(1)
            for dl in range(H_LOC):
                nc.sync.dma_start(wout_h[:, dl, :], woh_ap[:, dl, :])
                nc.sync.dma_start(wout_l[:, dl, :], wol_ap[:, dl, :])
            # --- proj(0) B-part: deferred (wl, xh) term + additive fixup ---
            for u in range(12):
                proj_unit(0, u, wide=True, terms='B')
            proj_with_attn(1, 0)
            stats(2)
            load_x(2)
            proj_with_attn(2, 1, filler=outproj_steps(0, range(NT)))
            stats(3)
            load_x(3)
            proj_with_attn(3, 2, filler=outproj_steps(1, range(NT)))
            emit_attn(3, filler=outproj_steps(2, range(NT), copy_eng='dve'))
            outproj_part(3, range(NT), wide=True)

    nc.finalize()
    return nc


_BUILT = None


def _get_nc():
    global _BUILT
    if _BUILT is None:
        _BUILT = _build()
    return _BUILT


E4 = ml_dtypes.float8_e4m3
BF = ml_dtypes.bfloat16


def _make_in_maps(x, norm_weight, w_qkv, w_out):
    x = np.asarray(x, dtype=np.float32)
    norm_weight = np.asarray(norm_weight, dtype=np.float32)
    w_qkv = np.asarray(w_qkv, dtype=np.float32) * norm_weight[None, :]
    w_out = np.asarray(w_out, dtype=np.float32)
    mask16 = np.triu(np.ones((128, 128), dtype=np.float32)).astype(BF)
    ones16 = np.ones((128, 8), dtype=np.float32).astype(BF)
    in_maps = []
    for core in range(8):
        b, g = divmod(core, 4)
        sl = slice(EL * g, EL * (g + 1))
        xs = x[b] * SX                      # [T, D]
        xq = xs.astype(E4)
        xl = (xs - xq.astype(np.float32)).astype(E4)
        wq = w_qkv[0 * D:1 * D][sl]
        wk = w_qkv[1 * D:2 * D][sl]
        wv = w_qkv[2 * D:3 * D][sl]
        wc = np.concatenate([wq, wk, wv], axis=0) * SW   # [1536, D]
        wh = wc.astype(E4)
        wl_ = (wc - wh.astype(np.float32)).astype(E4)
        wo = w_out[:, sl].T * SWO            # [EL, D]
        woh = wo.astype(E4)
        wol = (wo - woh.astype(np.float32)).astype(E4)
        in_maps.append({
            "xh8": np.ascontiguousarray(xq.T),
            "xtd8": np.ascontiguousarray(xq),
            "xl8": np.ascontiguousarray(xl.T),
            "wh8": np.ascontiguousarray(wh.T),
            "wl8": np.ascontiguousarray(wl_.T),
            "wouth8": np.ascontiguousarray(woh),
            "woutl8": np.ascontiguousarray(wol),
            "mask_in": mask16,
            "ones_in": ones16,
        })
    return in_maps


def _gather(results):
    out = np.zeros((B, T, D), dtype=np.float32)
    for core in range(8):
        b, _g = divmod(core, 4)
        out[b] += np.asarray(results[core]["outT"], dtype=np.float32).T
    return out


def run(x, norm_weight, w_qkv, w_out, trace=False):
    in_maps = _make_in_maps(x, norm_weight, w_qkv, w_out)
    if trace:
        try:
            res = run_bass_kernel_spmd(_get_nc(), in_maps, list(range(8)), trace=True)
            return _gather(res.results), res
        except Exception:
            pass  # NTFF hook unavailable under this axon client; run untraced
    res = run_bass_kernel_spmd(_get_nc(), in_maps, list(range(8)), trace=False)
    return _gather(res.results), res


def kernel(x, norm_weight, w_qkv, w_out):
    out, _res = run(x, norm_weight, w_qkv, w_out)
    return out
